# revision 19
# baseline (speedup 1.0000x reference)
"""Trainium2 Bass kernel: grouped-pointwise FFN with channel shuffle.

Computes (per batch b, all ops pointwise in T):
    h   = W1_grouped @ (x * mask) + b1          # G=4 block-diagonal GEMM
    h   = channel_shuffle(h, G)
    h   = gelu(h)                               # exact erf gelu
    out = (W2_grouped @ h + b2) * mask

Sharding: data-parallel over batch B=16 across 8 cores (2 batches/core).
Weights are replicated; no collectives.

Since the mask is a per-(b,t) scalar it commutes with the channel-dim
GEMMs, so both mask multiplies are folded to the host: x*mask before
upload, out*mask after download. All matmul operands are bf16 (host
cast); PSUM stays fp32; biases fp32.

Engine budget per core (warm): PE 131072 cycles @2.4GHz = 54.6us,
ScalarE gelu 65536 cols @1.2GHz + per-op overhead ~= 67us (the pacer),
DVE GEMM2-drain ~22us. Schedule: per (b,m) round, 8 slots of
[2 GEMM1 MMs N=512 -> ps1 [128,1024] -> gelu ACT N=1024], with the
previous round's GEMM2 (4 accumulating MMs + DVE bias-drain per 512-col
chunk) interleaved 2 MMs per slot so PE fills ScalarE's pacing gaps.
Channel shuffle is free: GEMM2's weights are pre-gathered on host so
group g2 contracts directly over GEMM1's (g, m=g2) tiles.

Warm-up: PE HAM needs ~3.4us of activity to unthrottle 1.2->2.4GHz;
a memset-sourced tile feeds warmup matmuls with no DMA dependency so
the PE is warm when the first x chunk lands.
"""

import numpy as np
import ml_dtypes

import concourse.mybir as mybir
import concourse.tile as tile
from concourse import bacc
from concourse import bass_utils

F32 = mybir.dt.float32
F32R = mybir.dt.float32r
BF16 = mybir.dt.bfloat16

N_CORES = 8
B, CIN, T = 16, 512, 2048
H, COUT, G = 2048, 512, 4
BPC = B // N_CORES        # batches per core
MB = (H // G) // 128      # 4 GEMM1 output-channel blocks per group
CH = 512                  # matmul free-dim chunk = 1 PSUM bank (fp32)
AW = 1024                 # gelu ACT width (2 PSUM banks)
N_WARMUP = 7              # N=512 warmup matmuls to warm the PE clock gate

MM_DT = BF16
# h / w2 run as float32r: same 1 col/cycle PE rate as bf16, but ACT and
# DVE write full 32-bit words (bf16 stores cost ~+160ns/op on both).
H_DT = F32R

_compiled = {}


def _build(mm_dt):
    nc = bacc.Bacc(
        "TRN2", target_bir_lowering=False, debug=False, num_devices=N_CORES
    )
    xs = nc.dram_tensor("xs", [BPC * G, 128, T], mm_dt, kind="ExternalInput").ap()
    # w1t columns are (m, g, o)-major; w2t columns are (g2, g, o2)-major
    # with the channel shuffle pre-applied (GEMM2 group g2 contracts
    # GEMM1's (g, m=g2) tiles).
    w1t = nc.dram_tensor("w1t", [128, G * MB * 128], mm_dt, kind="ExternalInput").ap()
    w2t = nc.dram_tensor("w2t", [128, G * G * 128], H_DT, kind="ExternalInput").ap()
    b1t = nc.dram_tensor("b1t", [128, G * MB], F32, kind="ExternalInput").ap()
    b2t = nc.dram_tensor("b2t", [128, G], F32, kind="ExternalInput").ap()
    outs = nc.dram_tensor("outs", [BPC * G, 128, T], BF16, kind="ExternalOutput").ap()

    with tile.TileContext(nc) as tc:
        with (
            tc.tile_pool(name="consts", bufs=1) as cpool,
            tc.tile_pool(name="xp", bufs=BPC * G) as xpool,
            tc.tile_pool(name="hp", bufs=2 * G) as hpool,
            tc.tile_pool(name="op", bufs=3) as opool,
            tc.tile_pool(name="ps1p", bufs=1, space="PSUM") as ps1pool,
            tc.tile_pool(name="ps2p", bufs=2, space="PSUM") as ps2pool,
        ):
            # One 6-bank PSUM tile, manually rotated in 3 x AW regions.
            # Tile's region/bank-aware tracker orders MM writes vs ACT
            # reads per region; adjacent regions let one ACT drain two
            # GEMM1 slots (N=2048) when they share (m, g) - same bias.
            ps1big = ps1pool.tile([128, 3 * AW], F32, tag="ps1", name="ps1big")
            # PE warm-up with no DMA dependency: memset a row, matmul on it.
            warm = cpool.tile([1, CH], mm_dt)
            nc.vector.memset(warm, 1.0)
            for _ in range(N_WARMUP):
                wps = ps2pool.tile([128, CH], F32, tag="ps2", name="wps")
                nc.tensor.matmul(wps, warm[:, 0:128], warm, start=True, stop=True)

            w1_sb = cpool.tile([128, G * MB * 128], mm_dt)
            w2_sb = cpool.tile([128, G * G * 128], H_DT)
            b1_sb = cpool.tile([128, G * MB], F32)
            b2_sb = cpool.tile([128, G], F32)
            x_sb = [[None] * G for _ in range(BPC)]

            def load_w1(m, ring):
                ws = slice(m * G * 128, (m + 1) * G * 128)
                ring.dma_start(w1_sb[:, ws], w1t[:, ws])

            def load_w2(g2, ring):
                ws = slice(g2 * G * 128, (g2 + 1) * G * 128)
                ring.dma_start(w2_sb[:, ws], w2t[:, ws])

            def alloc_x(b, g):
                x_sb[b][g] = xpool.tile([128, T], mm_dt, tag="x", name="xt")

            def load_x(b, g, ring, chunks, w=CH):
                for c in chunks:
                    cs = slice(c * w, (c + 1) * w)
                    ring.dma_start(x_sb[b][g][:, cs], xs[b * G + g][:, cs])

            # head: first half-T of batch0 on both rings (first tile in
            # two fine chunks so GEMM1 starts ~1us sooner), then the rest
            for g in range(G):
                alloc_x(0, g)
            # x halves split across rings in slot consumption order
            # (g-major); w2 follows on gpsimd just before round 1 needs it
            load_x(0, 0, nc.gpsimd, [0, 1])          # g0 h0 as 2 x 128KB
            load_w1(0, nc.sync)
            nc.sync.dma_start(b1_sb, b1t)
            nc.sync.dma_start(b2_sb, b2t)
            load_x(0, 1, nc.gpsimd, [0], w=AW)
            load_x(0, 0, nc.sync, [1], w=AW)
            load_x(0, 2, nc.gpsimd, [0], w=AW)
            load_x(0, 1, nc.sync, [1], w=AW)
            load_x(0, 3, nc.gpsimd, [0], w=AW)
            load_x(0, 2, nc.sync, [1], w=AW)
            load_w2(0, nc.gpsimd)
            load_x(0, 3, nc.sync, [1], w=AW)
            load_w1(1, nc.sync)
            load_w2(1, nc.gpsimd)
            load_w1(2, nc.sync)
            load_w2(2, nc.gpsimd)
            load_w1(3, nc.sync)
            load_w2(3, nc.gpsimd)

            def g1_mms(b, m, g, half, region):
                wap = w1_sb[:, (m * G + g) * 128 : (m * G + g + 1) * 128]
                base = half * AW
                rb = region * AW
                for k in range(AW // CH):
                    nc.tensor.matmul(
                        ps1big[:, rb + k * CH : rb + (k + 1) * CH],
                        wap,
                        x_sb[b][g][:, base + k * CH : base + (k + 1) * CH],
                        start=True, stop=True,
                    )

            def act(ht, m, g, half, region, width=AW):
                nc.scalar.activation(
                    ht[:, half * AW : half * AW + width],
                    ps1big[:, region * AW : region * AW + width],
                    mybir.ActivationFunctionType.Gelu,
                    bias=b1_sb[:, m * G + g : m * G + g + 1],
                    scale=1.0,
                )

            def gen_g2(b, g2, hts, ot, ring, fine=False):
                # yields once per GEMM2 matmul; drain + out-DMA ride along
                ob = b * G + g2
                for c in range(T // CH):
                    cs = slice(c * CH, (c + 1) * CH)
                    ps2t = ps2pool.tile([128, CH], F32, tag="ps2", name="ps2")
                    for g in range(G):
                        nc.tensor.matmul(
                            ps2t,
                            w2_sb[:, (g2 * G + g) * 128 : (g2 * G + g + 1) * 128],
                            hts[g][:, cs],
                            start=(g == 0), stop=(g == G - 1),
                        )
                        if g == G - 1:
                            nc.vector.tensor_scalar_add(
                                ot[:, cs], ps2t, b2_sb[:, g2 : g2 + 1]
                            )
                            if fine:
                                # final tiles: alternate rings so the last
                                # two transfers overlap
                                r = nc.sync if c % 2 == 0 else nc.gpsimd
                                r.dma_start(outs[ob][:, cs], ot[:, cs])
                            elif c % 2 == 1:
                                os_ = slice((c - 1) * CH, (c + 1) * CH)
                                ring.dma_start(outs[ob][:, os_], ot[:, os_])
                        yield

            def pump(gen, n):
                if gen is None:
                    return
                for _ in range(n):
                    try:
                        next(gen)
                    except StopIteration:
                        return

            rounds = [(b, m) for b in range(BPC) for m in range(MB)]
            prev = None
            gen = None
            lgen = None
            for ri, (b, m) in enumerate(rounds):
                is_last = ri == len(rounds) - 1
                hts = [hpool.tile([128, T], H_DT, tag="h", name="ht")
                       for _ in range(G)]
                if prev is not None:
                    ot = opool.tile([128, T], BF16, tag="o", name="ot")
                    gen = gen_g2(prev[0], prev[1], prev[2], ot,
                                 nc.sync if ri % 2 == 0 else nc.gpsimd)
                if is_last:
                    lot = opool.tile([128, T], BF16, tag="o", name="lot")
                    lgen = gen_g2(b, m, hts, lot, nc.sync, fine=True)
                if not is_last:
                    # (g, half) slot order; regions rotate 0,1,2 per round
                    # so pairs (g0, g2, g3) land on adjacent regions and
                    # drain in one N=2048 ACT.
                    for g in range(G):
                        ra = 2 * g % 3
                        rb = (2 * g + 1) % 3
                        paired = (rb == ra + 1) and (ri >= 1 or g >= 2)
                        g1_mms(b, m, g, 0, ra)
                        if not paired:
                            act(hts[g], m, g, 0, ra)
                        pump(gen, 2)
                        g1_mms(b, m, g, 1, rb)
                        if paired:
                            act(hts[g], m, g, 0, ra, width=2 * AW)
                        else:
                            act(hts[g], m, g, 1, rb)
                        pump(gen, 2)
                else:
                    # (half, g) order so chunks 0-1 of this round's own
                    # GEMM2 can interleave into half 1 (shorter tail).
                    for half in range(2):
                        for g in range(G):
                            r = (half * G + g) % 3
                            g1_mms(b, m, g, half, r)
                            act(hts[g], m, g, half, r)
                            pump(gen, 2)
                            if half == 1:
                                pump(lgen, 2)
                if ri == 1:
                    # prefetch batch 1 while round (0,1) computes
                    for g in range(G):
                        alloc_x(1, g)
                    load_x(1, 0, nc.gpsimd, [0], w=T)
                    load_x(1, 1, nc.gpsimd, [0], w=T)
                    load_x(1, 2, nc.sync, [0], w=T)
                    load_x(1, 3, nc.sync, [0], w=T)
                prev = (b, m, hts)
            pump(gen, 99)
            pump(lgen, 99)

    nc.compile()
    return nc


def get_nc(mm_dt=None):
    mm_dt = MM_DT if mm_dt is None else mm_dt
    if mm_dt not in _compiled:
        _compiled[mm_dt] = _build(mm_dt)
    return _compiled[mm_dt]


def prep_inputs(x, x_mask, w1, b1, w2, b2, mm_np=ml_dtypes.bfloat16):
    """Host-side layout prep. Returns per-core in_maps."""
    x = np.asarray(x, dtype=np.float32)
    x_mask = np.asarray(x_mask, dtype=np.float32)
    w1 = np.asarray(w1, dtype=np.float32)
    b1 = np.asarray(b1, dtype=np.float32)
    w2 = np.asarray(w2, dtype=np.float32)
    b2 = np.asarray(b2, dtype=np.float32)

    xm = x * x_mask  # input-side mask folded on host (commutes w/ GEMM)

    # w1 [H, CIN/G] -> lhsT blocks [i, (m, g, o)]
    w1r = w1.reshape(G, MB, 128, CIN // G)          # g, m, o, i
    w1t = np.ascontiguousarray(
        np.transpose(w1r, (3, 1, 0, 2)).reshape(128, G * MB * 128)
    ).astype(mm_np)
    # w2 [COUT, H/G] -> lhsT blocks [o, (g2, g, o2)], shuffle pre-applied
    # (stays fp32 bits: GEMM2 runs float32r)
    w2r = w2.reshape(G, 128, 128, G)                # g2, o2, r, g
    w2t = np.ascontiguousarray(
        np.transpose(w2r, (2, 0, 3, 1)).reshape(128, G * G * 128)
    )
    b1tt = np.ascontiguousarray(
        b1.reshape(G, MB, 128).transpose(2, 1, 0).reshape(128, G * MB)
    )
    b2tt = np.ascontiguousarray(b2.reshape(G, 128).T)

    xr = xm.reshape(N_CORES, BPC * G, 128, T).astype(mm_np)

    in_maps = []
    for k in range(N_CORES):
        in_maps.append(
            {
                "xs": np.ascontiguousarray(xr[k]),
                "w1t": w1t,
                "w2t": w2t,
                "b1t": b1tt,
                "b2t": b2tt,
            }
        )
    return in_maps


def assemble_output(results, x_mask):
    """results: list of 8 dicts with 'outs' [BPC*G, 128, T] (fp32)."""
    parts = [
        np.asarray(r["outs"]).astype(np.float32).reshape(BPC, G * 128, T)
        for r in results
    ]
    out = np.concatenate(parts, axis=0)
    return out * np.asarray(x_mask, dtype=np.float32)


def kernel(x, x_mask, w1, b1, w2, b2, n_groups):
    assert int(n_groups) == G
    import os

    # NTFF tracing needs antenv.axon_hooks, absent on this image; make
    # sure an inherited BASS_TRACE can't push us onto that path.
    os.environ["BASS_NEVER_TRACE"] = "1"
    nc = get_nc()
    mm_np = np.float32 if MM_DT in (F32, mybir.dt.float32r) else ml_dtypes.bfloat16
    in_maps = prep_inputs(x, x_mask, w1, b1, w2, b2, mm_np=mm_np)
    res = bass_utils.run_bass_kernel_spmd(
        nc, in_maps, core_ids=list(range(N_CORES))
    )
    return assemble_output(res.results, x_mask)


# revision 23
# speedup vs baseline: 1.4617x; 1.4617x over previous
"""Trainium2 Bass kernel: grouped-pointwise FFN with channel shuffle.

Computes (per batch b, all ops pointwise in T):
    h   = W1_grouped @ (x * mask) + b1          # G=4 block-diagonal GEMM
    h   = channel_shuffle(h, G)
    h   = gelu(h)                               # exact erf gelu
    out = (W2_grouped @ h + b2) * mask

Sharding: data-parallel over batch B=16 across 8 cores (2 batches/core).
Weights are replicated; no collectives.

Since the mask is a per-(b,t) scalar it commutes with the channel-dim
GEMMs, so both mask multiplies are folded to the host: x*mask before
upload, out*mask after download. All matmul operands are bf16 (host
cast); PSUM stays fp32; biases fp32.

Engine budget per core (warm): PE 131072 cycles @2.4GHz = 54.6us,
ScalarE gelu 65536 cols @1.2GHz + per-op overhead ~= 67us (the pacer),
DVE GEMM2-drain ~22us. Schedule: per (b,m) round, 8 slots of
[2 GEMM1 MMs N=512 -> ps1 [128,1024] -> gelu ACT N=1024], with the
previous round's GEMM2 (4 accumulating MMs + DVE bias-drain per 512-col
chunk) interleaved 2 MMs per slot so PE fills ScalarE's pacing gaps.
Channel shuffle is free: GEMM2's weights are pre-gathered on host so
group g2 contracts directly over GEMM1's (g, m=g2) tiles.

Warm-up: PE HAM needs ~3.4us of activity to unthrottle 1.2->2.4GHz;
a memset-sourced tile feeds warmup matmuls with no DMA dependency so
the PE is warm when the first x chunk lands.
"""

import numpy as np
import ml_dtypes

import concourse.mybir as mybir
import concourse.tile as tile
from concourse import bacc
from concourse import bass_utils

F32 = mybir.dt.float32
F32R = mybir.dt.float32r
BF16 = mybir.dt.bfloat16

N_CORES = 8
B, CIN, T = 16, 512, 2048
H, COUT, G = 2048, 512, 4
BPC = B // N_CORES        # batches per core
MB = (H // G) // 128      # 4 GEMM1 output-channel blocks per group
CH = 512                  # matmul free-dim chunk = 1 PSUM bank (fp32)
AW = 1024                 # gelu ACT width (2 PSUM banks)
N_WARMUP = 7              # N=512 warmup matmuls to warm the PE clock gate

MM_DT = BF16
# h / w2 run as float32r: same 1 col/cycle PE rate as bf16, but ACT and
# DVE write full 32-bit words (bf16 stores cost ~+160ns/op on both).
H_DT = F32R

_compiled = {}


def _build(mm_dt):
    nc = bacc.Bacc(
        "TRN2", target_bir_lowering=False, debug=False, num_devices=N_CORES
    )
    xs = nc.dram_tensor("xs", [BPC * G, 128, T], mm_dt, kind="ExternalInput").ap()
    # w1t columns are (m, g, o)-major; w2t columns are (g2, g, o2)-major
    # with the channel shuffle pre-applied (GEMM2 group g2 contracts
    # GEMM1's (g, m=g2) tiles).
    w1t = nc.dram_tensor("w1t", [128, G * MB * 128], mm_dt, kind="ExternalInput").ap()
    w2t = nc.dram_tensor("w2t", [128, G * G * 128], H_DT, kind="ExternalInput").ap()
    b1t = nc.dram_tensor("b1t", [128, G * MB], F32, kind="ExternalInput").ap()
    b2t = nc.dram_tensor("b2t", [128, G], F32, kind="ExternalInput").ap()
    outs = nc.dram_tensor("outs", [BPC * G, 128, T], BF16, kind="ExternalOutput").ap()

    with tile.TileContext(nc) as tc:
        with (
            tc.tile_pool(name="consts", bufs=1) as cpool,
            tc.tile_pool(name="xp", bufs=BPC * G) as xpool,
            tc.tile_pool(name="hp", bufs=2 * G) as hpool,
            tc.tile_pool(name="op", bufs=3) as opool,
            tc.tile_pool(name="ps1p", bufs=3, space="PSUM") as ps1pool,
            tc.tile_pool(name="ps2p", bufs=2, space="PSUM") as ps2pool,
        ):
            # PE warm-up with no DMA dependency: memset a row, matmul on it.
            warm = cpool.tile([1, CH], mm_dt)
            nc.vector.memset(warm, 1.0)
            for _ in range(N_WARMUP):
                wps = ps2pool.tile([128, CH], F32, tag="ps2", name="wps")
                nc.tensor.matmul(wps, warm[:, 0:128], warm, start=True, stop=True)

            w1_sb = cpool.tile([128, G * MB * 128], mm_dt)
            w2_sb = cpool.tile([128, G * G * 128], H_DT)
            b1_sb = cpool.tile([128, G * MB], F32)
            b2_sb = cpool.tile([128, G], F32)
            x_sb = [[None] * G for _ in range(BPC)]

            def load_w1(m, ring):
                ws = slice(m * G * 128, (m + 1) * G * 128)
                ring.dma_start(w1_sb[:, ws], w1t[:, ws])

            def load_w2(g2, ring):
                ws = slice(g2 * G * 128, (g2 + 1) * G * 128)
                ring.dma_start(w2_sb[:, ws], w2t[:, ws])

            def alloc_x(b, g):
                x_sb[b][g] = xpool.tile([128, T], mm_dt, tag="x", name="xt")

            def load_x(b, g, ring, chunks, w=CH):
                for c in chunks:
                    cs = slice(c * w, (c + 1) * w)
                    ring.dma_start(x_sb[b][g][:, cs], xs[b * G + g][:, cs])

            # head: first half-T of batch0 on both rings (first tile in
            # two fine chunks so GEMM1 starts ~1us sooner), then the rest
            for g in range(G):
                alloc_x(0, g)
            # (half, g) consumption order: h0 of all g first, then h1;
            # b1/b2 early so the first ACT isn't bias-gated; w2 on the
            # otherwise-idle gpsimd ring just before round 1 needs it.
            load_x(0, 0, nc.gpsimd, [0, 1])          # g0 h0 as 2 x 128KB
            load_w1(0, nc.sync)
            nc.sync.dma_start(b1_sb, b1t)
            nc.sync.dma_start(b2_sb, b2t)
            load_x(0, 2, nc.sync, [0], w=AW)
            load_x(0, 1, nc.gpsimd, [0], w=AW)
            load_x(0, 3, nc.sync, [0], w=AW)
            load_x(0, 0, nc.gpsimd, [1], w=AW)
            load_x(0, 1, nc.gpsimd, [1], w=AW)
            load_x(0, 2, nc.sync, [1], w=AW)
            load_x(0, 3, nc.sync, [1], w=AW)
            load_w1(1, nc.sync)
            load_w2(0, nc.gpsimd)
            load_w1(2, nc.sync)
            load_w2(1, nc.gpsimd)
            load_w1(3, nc.sync)
            load_w2(2, nc.gpsimd)
            load_w2(3, nc.gpsimd)

            def g1_slot(b, m, g, half, ht):
                # one ps1 tile [128, AW]: 2 matmuls + fused gelu(+b1) drain
                ps1t = ps1pool.tile([128, AW], F32, tag="ps1", name="ps1")
                wap = w1_sb[:, (m * G + g) * 128 : (m * G + g + 1) * 128]
                base = half * AW
                for k in range(AW // CH):
                    nc.tensor.matmul(
                        ps1t[:, k * CH : (k + 1) * CH],
                        wap,
                        x_sb[b][g][:, base + k * CH : base + (k + 1) * CH],
                        start=True, stop=True,
                    )
                nc.scalar.activation(
                    ht[:, base : base + AW],
                    ps1t,
                    mybir.ActivationFunctionType.Gelu,
                    bias=b1_sb[:, m * G + g : m * G + g + 1],
                    scale=1.0,
                )

            def gen_g2(b, g2, hts, ot, ring, fine=False):
                # yields once per GEMM2 matmul; drain + out-DMA ride along
                ob = b * G + g2
                for c in range(T // CH):
                    cs = slice(c * CH, (c + 1) * CH)
                    ps2t = ps2pool.tile([128, CH], F32, tag="ps2", name="ps2")
                    for g in range(G):
                        nc.tensor.matmul(
                            ps2t,
                            w2_sb[:, (g2 * G + g) * 128 : (g2 * G + g + 1) * 128],
                            hts[g][:, cs],
                            start=(g == 0), stop=(g == G - 1),
                        )
                        if g == G - 1:
                            nc.vector.tensor_scalar_add(
                                ot[:, cs], ps2t, b2_sb[:, g2 : g2 + 1]
                            )
                            if fine:
                                # final tiles: alternate rings so the last
                                # two transfers overlap
                                r = nc.sync if c % 2 == 0 else nc.gpsimd
                                r.dma_start(outs[ob][:, cs], ot[:, cs])
                            elif c % 2 == 1:
                                os_ = slice((c - 1) * CH, (c + 1) * CH)
                                ring.dma_start(outs[ob][:, os_], ot[:, os_])
                        yield

            def pump(gen, n):
                if gen is None:
                    return
                for _ in range(n):
                    try:
                        next(gen)
                    except StopIteration:
                        return

            rounds = [(b, m) for b in range(BPC) for m in range(MB)]
            prev = None
            gen = None
            lgen = None
            for ri, (b, m) in enumerate(rounds):
                is_last = ri == len(rounds) - 1
                hts = [hpool.tile([128, T], H_DT, tag="h", name="ht")
                       for _ in range(G)]
                if prev is not None:
                    ot = opool.tile([128, T], BF16, tag="o", name="ot")
                    gen = gen_g2(prev[0], prev[1], prev[2], ot,
                                 nc.sync if ri % 2 == 0 else nc.gpsimd)
                if is_last:
                    lot = opool.tile([128, T], BF16, tag="o", name="lot")
                    lgen = gen_g2(b, m, hts, lot, nc.sync, fine=True)
                for half in range(2):
                    for g in range(G):
                        g1_slot(b, m, g, half, hts[g])
                        pump(gen, 2)
                        if is_last and half == 1:
                            # this round's own GEMM2 chunks 0-1 (cols
                            # 0:1024, produced by half-0 ACTs) interleave
                            # here; chunks 2-3 wait for half-1 ACTs.
                            pump(lgen, 2)
                if ri == 1:
                    # prefetch batch 1 while round (0,1) computes
                    for g in range(G):
                        alloc_x(1, g)
                    load_x(1, 0, nc.gpsimd, [0], w=T)
                    load_x(1, 1, nc.gpsimd, [0], w=T)
                    load_x(1, 2, nc.sync, [0], w=T)
                    load_x(1, 3, nc.sync, [0], w=T)
                prev = (b, m, hts)
            pump(gen, 99)
            pump(lgen, 99)

    nc.compile()
    return nc


def get_nc(mm_dt=None):
    mm_dt = MM_DT if mm_dt is None else mm_dt
    if mm_dt not in _compiled:
        _compiled[mm_dt] = _build(mm_dt)
    return _compiled[mm_dt]


def prep_inputs(x, x_mask, w1, b1, w2, b2, mm_np=ml_dtypes.bfloat16):
    """Host-side layout prep. Returns per-core in_maps."""
    x = np.asarray(x, dtype=np.float32)
    x_mask = np.asarray(x_mask, dtype=np.float32)
    w1 = np.asarray(w1, dtype=np.float32)
    b1 = np.asarray(b1, dtype=np.float32)
    w2 = np.asarray(w2, dtype=np.float32)
    b2 = np.asarray(b2, dtype=np.float32)

    xm = x * x_mask  # input-side mask folded on host (commutes w/ GEMM)

    # w1 [H, CIN/G] -> lhsT blocks [i, (m, g, o)]
    w1r = w1.reshape(G, MB, 128, CIN // G)          # g, m, o, i
    w1t = np.ascontiguousarray(
        np.transpose(w1r, (3, 1, 0, 2)).reshape(128, G * MB * 128)
    ).astype(mm_np)
    # w2 [COUT, H/G] -> lhsT blocks [o, (g2, g, o2)], shuffle pre-applied
    # (stays fp32 bits: GEMM2 runs float32r)
    w2r = w2.reshape(G, 128, 128, G)                # g2, o2, r, g
    w2t = np.ascontiguousarray(
        np.transpose(w2r, (2, 0, 3, 1)).reshape(128, G * G * 128)
    )
    b1tt = np.ascontiguousarray(
        b1.reshape(G, MB, 128).transpose(2, 1, 0).reshape(128, G * MB)
    )
    b2tt = np.ascontiguousarray(b2.reshape(G, 128).T)

    xr = xm.reshape(N_CORES, BPC * G, 128, T).astype(mm_np)

    in_maps = []
    for k in range(N_CORES):
        in_maps.append(
            {
                "xs": np.ascontiguousarray(xr[k]),
                "w1t": w1t,
                "w2t": w2t,
                "b1t": b1tt,
                "b2t": b2tt,
            }
        )
    return in_maps


def assemble_output(results, x_mask):
    """results: list of 8 dicts with 'outs' [BPC*G, 128, T] (fp32)."""
    parts = [
        np.asarray(r["outs"]).astype(np.float32).reshape(BPC, G * 128, T)
        for r in results
    ]
    out = np.concatenate(parts, axis=0)
    return out * np.asarray(x_mask, dtype=np.float32)


def kernel(x, x_mask, w1, b1, w2, b2, n_groups):
    assert int(n_groups) == G
    import os

    # NTFF tracing needs antenv.axon_hooks, absent on this image; make
    # sure an inherited BASS_TRACE can't push us onto that path.
    os.environ["BASS_NEVER_TRACE"] = "1"
    nc = get_nc()
    mm_np = np.float32 if MM_DT in (F32, mybir.dt.float32r) else ml_dtypes.bfloat16
    in_maps = prep_inputs(x, x_mask, w1, b1, w2, b2, mm_np=mm_np)
    res = bass_utils.run_bass_kernel_spmd(
        nc, in_maps, core_ids=list(range(N_CORES))
    )
    return assemble_output(res.results, x_mask)


# revision 25
# speedup vs baseline: 1.4761x; 1.0099x over previous
"""Trainium2 Bass kernel: grouped-pointwise FFN with channel shuffle.

Computes (per batch b, all ops pointwise in T):
    h   = W1_grouped @ (x * mask) + b1          # G=4 block-diagonal GEMM
    h   = channel_shuffle(h, G)
    h   = gelu(h)                               # exact erf gelu
    out = (W2_grouped @ h + b2) * mask

Sharding: data-parallel over batch B=16 across 8 cores (2 batches/core).
Weights are replicated; no collectives.

Since the mask is a per-(b,t) scalar it commutes with the channel-dim
GEMMs, so both mask multiplies are folded to the host: x*mask before
upload, out*mask after download. All matmul operands are bf16 (host
cast); PSUM stays fp32; biases fp32.

Engine budget per core (warm): PE 131072 cycles @2.4GHz = 54.6us,
ScalarE gelu 65536 cols @1.2GHz + per-op overhead ~= 67us (the pacer),
DVE GEMM2-drain ~22us. Schedule: per (b,m) round, 8 slots of
[2 GEMM1 MMs N=512 -> ps1 [128,1024] -> gelu ACT N=1024], with the
previous round's GEMM2 (4 accumulating MMs + DVE bias-drain per 512-col
chunk) interleaved 2 MMs per slot so PE fills ScalarE's pacing gaps.
Channel shuffle is free: GEMM2's weights are pre-gathered on host so
group g2 contracts directly over GEMM1's (g, m=g2) tiles.

Warm-up: PE HAM needs ~3.4us of activity to unthrottle 1.2->2.4GHz;
a memset-sourced tile feeds warmup matmuls with no DMA dependency so
the PE is warm when the first x chunk lands.
"""

import numpy as np
import ml_dtypes

import concourse.mybir as mybir
import concourse.tile as tile
from concourse import bacc
from concourse import bass_utils

F32 = mybir.dt.float32
F32R = mybir.dt.float32r
BF16 = mybir.dt.bfloat16

N_CORES = 8
B, CIN, T = 16, 512, 2048
H, COUT, G = 2048, 512, 4
BPC = B // N_CORES        # batches per core
MB = (H // G) // 128      # 4 GEMM1 output-channel blocks per group
CH = 512                  # matmul free-dim chunk = 1 PSUM bank (fp32)
AW = 1024                 # gelu ACT width (2 PSUM banks)
N_WARMUP = 7              # N=512 warmup matmuls to warm the PE clock gate

MM_DT = BF16
# h / w2 run as float32r: same 1 col/cycle PE rate as bf16, but ACT and
# DVE write full 32-bit words (bf16 stores cost ~+160ns/op on both).
H_DT = F32R

_compiled = {}


def _build(mm_dt):
    nc = bacc.Bacc(
        "TRN2", target_bir_lowering=False, debug=False, num_devices=N_CORES
    )
    xs = nc.dram_tensor("xs", [BPC * G, 128, T], mm_dt, kind="ExternalInput").ap()
    # w1t columns are (m, g, o)-major; w2t columns are (g2, g, o2)-major
    # with the channel shuffle pre-applied (GEMM2 group g2 contracts
    # GEMM1's (g, m=g2) tiles).
    w1t = nc.dram_tensor("w1t", [128, G * MB * 128], mm_dt, kind="ExternalInput").ap()
    w2t = nc.dram_tensor("w2t", [128, G * G * 128], H_DT, kind="ExternalInput").ap()
    b1t = nc.dram_tensor("b1t", [128, G * MB], F32, kind="ExternalInput").ap()
    b2t = nc.dram_tensor("b2t", [128, G], F32, kind="ExternalInput").ap()
    outs = nc.dram_tensor("outs", [BPC * G, 128, T], BF16, kind="ExternalOutput").ap()

    with tile.TileContext(nc) as tc:
        with (
            tc.tile_pool(name="consts", bufs=1) as cpool,
            tc.tile_pool(name="xp", bufs=BPC * G) as xpool,
            tc.tile_pool(name="hp", bufs=2 * G) as hpool,
            tc.tile_pool(name="op", bufs=3) as opool,
            tc.tile_pool(name="ps1p", bufs=3, space="PSUM") as ps1pool,
            tc.tile_pool(name="ps2p", bufs=2, space="PSUM") as ps2pool,
        ):
            # PE warm-up with no DMA dependency: memset a row, matmul on it.
            warm = cpool.tile([1, CH], mm_dt)
            nc.vector.memset(warm, 1.0)
            for _ in range(N_WARMUP):
                wps = ps2pool.tile([128, CH], F32, tag="ps2", name="wps")
                nc.tensor.matmul(wps, warm[:, 0:128], warm, start=True, stop=True)

            w1_sb = cpool.tile([128, G * MB * 128], mm_dt)
            w2_sb = cpool.tile([128, G * G * 128], H_DT)
            b1_sb = cpool.tile([128, G * MB], F32)
            b2_sb = cpool.tile([128, G], F32)
            x_sb = [[None] * G for _ in range(BPC)]

            def load_w1(m, ring):
                ws = slice(m * G * 128, (m + 1) * G * 128)
                ring.dma_start(w1_sb[:, ws], w1t[:, ws])

            def load_w2(g2, ring):
                ws = slice(g2 * G * 128, (g2 + 1) * G * 128)
                ring.dma_start(w2_sb[:, ws], w2t[:, ws])

            def alloc_x(b, g):
                x_sb[b][g] = xpool.tile([128, T], mm_dt, tag="x", name="xt")

            def load_x(b, g, ring, chunks, w=CH):
                for c in chunks:
                    cs = slice(c * w, (c + 1) * w)
                    ring.dma_start(x_sb[b][g][:, cs], xs[b * G + g][:, cs])

            # head: first half-T of batch0 on both rings (first tile in
            # two fine chunks so GEMM1 starts ~1us sooner), then the rest
            for g in range(G):
                alloc_x(0, g)
            # head: first half-T of batch0 on both rings (first tile in
            # two fine chunks so GEMM1 starts ~1us sooner), then the rest
            load_x(0, 0, nc.gpsimd, [0, 1])          # 2 x 128KB
            load_w1(0, nc.sync)
            load_x(0, 2, nc.sync, [0], w=AW)
            load_x(0, 1, nc.gpsimd, [0], w=AW)
            nc.sync.dma_start(b1_sb, b1t)
            nc.sync.dma_start(b2_sb, b2t)
            load_x(0, 3, nc.sync, [0], w=AW)
            load_x(0, 0, nc.gpsimd, [1], w=AW)
            load_x(0, 1, nc.gpsimd, [1], w=AW)
            load_w1(1, nc.sync)
            load_x(0, 2, nc.sync, [1], w=AW)
            load_x(0, 3, nc.sync, [1], w=AW)
            load_w1(2, nc.sync)
            load_w1(3, nc.sync)
            # w2 on the otherwise-idle gpsimd ring, just-in-time:
            # block g2 is first used in round g2+1.
            load_w2(0, nc.gpsimd)
            load_w2(1, nc.gpsimd)

            def g1_slot(b, m, g, half, ht):
                # one ps1 tile [128, AW]: 2 matmuls + fused gelu(+b1) drain
                ps1t = ps1pool.tile([128, AW], F32, tag="ps1", name="ps1")
                wap = w1_sb[:, (m * G + g) * 128 : (m * G + g + 1) * 128]
                base = half * AW
                for k in range(AW // CH):
                    nc.tensor.matmul(
                        ps1t[:, k * CH : (k + 1) * CH],
                        wap,
                        x_sb[b][g][:, base + k * CH : base + (k + 1) * CH],
                        start=True, stop=True,
                    )
                nc.scalar.activation(
                    ht[:, base : base + AW],
                    ps1t,
                    mybir.ActivationFunctionType.Gelu,
                    bias=b1_sb[:, m * G + g : m * G + g + 1],
                    scale=1.0,
                )

            def gen_g2(b, g2, hts, ot, ring, fine=False):
                # yields once per GEMM2 matmul; drain + out-DMA ride along
                ob = b * G + g2
                for c in range(T // CH):
                    cs = slice(c * CH, (c + 1) * CH)
                    ps2t = ps2pool.tile([128, CH], F32, tag="ps2", name="ps2")
                    for g in range(G):
                        nc.tensor.matmul(
                            ps2t,
                            w2_sb[:, (g2 * G + g) * 128 : (g2 * G + g + 1) * 128],
                            hts[g][:, cs],
                            start=(g == 0), stop=(g == G - 1),
                        )
                        if g == G - 1:
                            nc.vector.tensor_scalar_add(
                                ot[:, cs], ps2t, b2_sb[:, g2 : g2 + 1]
                            )
                            if fine:
                                # final tiles: alternate rings so the last
                                # two transfers overlap
                                r = nc.sync if c % 2 == 0 else nc.gpsimd
                                r.dma_start(outs[ob][:, cs], ot[:, cs])
                            elif c % 2 == 1:
                                os_ = slice((c - 1) * CH, (c + 1) * CH)
                                ring.dma_start(outs[ob][:, os_], ot[:, os_])
                        yield

            def pump(gen, n):
                if gen is None:
                    return
                for _ in range(n):
                    try:
                        next(gen)
                    except StopIteration:
                        return

            rounds = [(b, m) for b in range(BPC) for m in range(MB)]
            prev = None
            gen = None
            lgen = None
            for ri, (b, m) in enumerate(rounds):
                is_last = ri == len(rounds) - 1
                hts = [hpool.tile([128, T], H_DT, tag="h", name="ht")
                       for _ in range(G)]
                if prev is not None:
                    ot = opool.tile([128, T], BF16, tag="o", name="ot")
                    gen = gen_g2(prev[0], prev[1], prev[2], ot,
                                 nc.sync if ri % 2 == 0 else nc.gpsimd)
                if is_last:
                    lot = opool.tile([128, T], BF16, tag="o", name="lot")
                    lgen = gen_g2(b, m, hts, lot, nc.sync, fine=True)
                for half in range(2):
                    for g in range(G):
                        g1_slot(b, m, g, half, hts[g])
                        pump(gen, 2)
                        if is_last and half == 1:
                            # this round's own GEMM2 chunks 0-1 (cols
                            # 0:1024, produced by half-0 ACTs) interleave
                            # here; chunks 2-3 wait for half-1 ACTs.
                            pump(lgen, 2)
                if ri == 0:
                    load_w2(2, nc.gpsimd)
                    load_w2(3, nc.gpsimd)
                if ri == 1:
                    # prefetch batch 1 while round (0,1) computes
                    for g in range(G):
                        alloc_x(1, g)
                    load_x(1, 0, nc.gpsimd, [0], w=T)
                    load_x(1, 1, nc.gpsimd, [0], w=T)
                    load_x(1, 2, nc.sync, [0], w=T)
                    load_x(1, 3, nc.sync, [0], w=T)
                prev = (b, m, hts)
            pump(gen, 99)
            pump(lgen, 99)

    nc.compile()
    return nc


def get_nc(mm_dt=None):
    mm_dt = MM_DT if mm_dt is None else mm_dt
    if mm_dt not in _compiled:
        _compiled[mm_dt] = _build(mm_dt)
    return _compiled[mm_dt]


def prep_inputs(x, x_mask, w1, b1, w2, b2, mm_np=ml_dtypes.bfloat16):
    """Host-side layout prep. Returns per-core in_maps."""
    x = np.asarray(x, dtype=np.float32)
    x_mask = np.asarray(x_mask, dtype=np.float32)
    w1 = np.asarray(w1, dtype=np.float32)
    b1 = np.asarray(b1, dtype=np.float32)
    w2 = np.asarray(w2, dtype=np.float32)
    b2 = np.asarray(b2, dtype=np.float32)

    xm = x * x_mask  # input-side mask folded on host (commutes w/ GEMM)

    # w1 [H, CIN/G] -> lhsT blocks [i, (m, g, o)]
    w1r = w1.reshape(G, MB, 128, CIN // G)          # g, m, o, i
    w1t = np.ascontiguousarray(
        np.transpose(w1r, (3, 1, 0, 2)).reshape(128, G * MB * 128)
    ).astype(mm_np)
    # w2 [COUT, H/G] -> lhsT blocks [o, (g2, g, o2)], shuffle pre-applied
    # (stays fp32 bits: GEMM2 runs float32r)
    w2r = w2.reshape(G, 128, 128, G)                # g2, o2, r, g
    w2t = np.ascontiguousarray(
        np.transpose(w2r, (2, 0, 3, 1)).reshape(128, G * G * 128)
    )
    b1tt = np.ascontiguousarray(
        b1.reshape(G, MB, 128).transpose(2, 1, 0).reshape(128, G * MB)
    )
    b2tt = np.ascontiguousarray(b2.reshape(G, 128).T)

    xr = xm.reshape(N_CORES, BPC * G, 128, T).astype(mm_np)

    in_maps = []
    for k in range(N_CORES):
        in_maps.append(
            {
                "xs": np.ascontiguousarray(xr[k]),
                "w1t": w1t,
                "w2t": w2t,
                "b1t": b1tt,
                "b2t": b2tt,
            }
        )
    return in_maps


def assemble_output(results, x_mask):
    """results: list of 8 dicts with 'outs' [BPC*G, 128, T] (fp32)."""
    parts = [
        np.asarray(r["outs"]).astype(np.float32).reshape(BPC, G * 128, T)
        for r in results
    ]
    out = np.concatenate(parts, axis=0)
    return out * np.asarray(x_mask, dtype=np.float32)


def kernel(x, x_mask, w1, b1, w2, b2, n_groups):
    assert int(n_groups) == G
    import os

    # NTFF tracing needs antenv.axon_hooks, absent on this image; make
    # sure an inherited BASS_TRACE can't push us onto that path.
    os.environ["BASS_NEVER_TRACE"] = "1"
    nc = get_nc()
    mm_np = np.float32 if MM_DT in (F32, mybir.dt.float32r) else ml_dtypes.bfloat16
    in_maps = prep_inputs(x, x_mask, w1, b1, w2, b2, mm_np=mm_np)
    res = bass_utils.run_bass_kernel_spmd(
        nc, in_maps, core_ids=list(range(N_CORES))
    )
    return assemble_output(res.results, x_mask)
